# revision 6
# baseline (speedup 1.0000x reference)
"""CQAttention (BiDAF context-query attention) Trainium2 kernel, v2.

Shapes: C (32,128,1024), Q (32,128,512), W (32768,1,384) -> out (32,512,1024).
Data-parallel across 8 NeuronCores: 4 batches per core, no collectives.

v2 strategy (vs v1):
  - All PE matmuls in bf16 (2x column rate vs fp32r).
  - Zero PE transposes: host supplies W^T/Q^T/C^T layouts (pure layout prep).
  - F = exp(S^T) == E^T exactly (the per-context softmax bias r cancels in
    the row normalization), so the second exp pass and the S^T matmuls are
    replaced by a DMA xbar transpose of E (SBUF->SBUF, no engine cycles).
  - A^T/B^T scaling row 1/totF via a ones-vector matmul over F, broadcast
    across partitions with gpsimd.partition_broadcast.
  - Elementwise work split between DVE and GPSIMD.

Per-batch algorithm (d-major tiles, 128 partitions):
  UT[d,c]  = wq^T + wqc^T * C                      (GPSIMD)
  rbias[c] = sum_d wc*C^T                          (DVE, c-part chunks)
  S chunk  = UT_k^T @ Q ; E = exp(S + rbias)       (PE + ACT, bf16 out)
  F        = E^T                                   (DMA xbar)
  G        = E^T @ [C^T | 1] per q-chunk           (PE accum) -> Gn = G/colsum
  totF[c]  = ones^T @ F ; rr = 1/totF              (PE + DVE) -> rrB broadcast
  A^T      = (Qt @ F) * rrB ; CA = C * A^T         (PE + DVE/GPSIMD)
  CB       = (Gn @ F) * rrB * C                    (PE + DVE/GPSIMD)
  out      = [C ; A^T ; CA ; CB]
"""

import ml_dtypes
import numpy as np

import concourse.bass as bass
import concourse.bacc as bacc
import concourse.mybir as mybir
from concourse import tile
from concourse.bass_utils import run_bass_kernel_spmd

B, D, CL, QL = 32, 128, 1024, 512
NCORES = 8
BPC = B // NCORES          # batches per core
NK = CL // D               # 8 c-chunks of 128
NJ = QL // D               # 4 q-chunks of 128

F32 = mybir.dt.float32
BF16 = mybir.dt.bfloat16
EXP = mybir.ActivationFunctionType.Exp
BF = ml_dtypes.bfloat16

_NC = None
RUN_KWARGS = {}        # test harness can set e.g. {"trace": True}
LAST_RESULT = None     # last BassKernelResults (for exec_time_ns / trace)


def _build():
    nc = bacc.Bacc("TRN2", debug=False, num_devices=NCORES)

    C_d = nc.dram_tensor("C", [BPC, D, CL], F32, kind="ExternalInput").ap()
    QB_d = nc.dram_tensor("QB", [BPC, D, QL], BF16, kind="ExternalInput").ap()
    QT_d = nc.dram_tensor("QT", [BPC, D, NJ * D], BF16, kind="ExternalInput").ap()
    WQT_d = nc.dram_tensor("WQT", [BPC, D, CL], BF16, kind="ExternalInput").ap()
    WQCT_d = nc.dram_tensor("WQCT", [BPC, D, CL], BF16, kind="ExternalInput").ap()
    WC_d = nc.dram_tensor("WC", [BPC, D, NK * D], BF16, kind="ExternalInput").ap()
    CTO_d = nc.dram_tensor("CTO", [BPC, D, NK * (D + 1)], BF16,
                           kind="ExternalInput").ap()
    OUT_d = nc.dram_tensor("OUT", [BPC, 4 * D, CL], F32, kind="ExternalOutput").ap()

    with tile.TileContext(nc) as tc:
        with (
            tc.tile_pool(name="const", bufs=1) as cpool,
            tc.tile_pool(name="work", bufs=2) as pool,
            tc.tile_pool(name="psS", bufs=3, space="PSUM") as psS,
            tc.tile_pool(name="psG", bufs=2, space="PSUM") as psG,
            tc.tile_pool(name="psAB", bufs=2, space="PSUM") as psAB,
            tc.tile_pool(name="psRow", bufs=1, space="PSUM") as psRow,
        ):
            onesq = cpool.tile([D, 1], BF16)
            nc.vector.memset(onesq[:], 1.0)
            pools = (pool, psS, psG, psAB, psRow)
            for b in range(BPC):
                _batch(nc, tc, pools, onesq,
                       C_d[b], QB_d[b], QT_d[b], WQT_d[b], WQCT_d[b],
                       WC_d[b], CTO_d[b], OUT_d[b])
    nc.compile()
    return nc


def _batch(nc, tc, pools, onesq, C_d, QB_d, QT_d, WQT_d, WQCT_d, WC_d, CTO_d,
           OUT_d):
    pool, psS, psG, psAB, psRow = pools

    # ---- input loads (sync HWDGE ring) ----
    Ctile = pool.tile([D, CL], F32, tag="Ctile")
    Qb = pool.tile([D, QL], BF16, tag="Qb")
    Qt = pool.tile([D, NJ * D], BF16, tag="Qt")
    wqT = pool.tile([D, CL], BF16, tag="wqT")
    wqcT = pool.tile([D, CL], BF16, tag="wqcT")
    wc = pool.tile([D, NK * D], BF16, tag="wc")
    cto = pool.tile([D, NK * (D + 1)], BF16, tag="cto")
    nc.sync.dma_start(Ctile[:], C_d[:])
    nc.sync.dma_start(Qb[:], QB_d[:])
    nc.sync.dma_start(Qt[:], QT_d[:])
    nc.sync.dma_start(wqT[:], WQT_d[:])
    nc.sync.dma_start(wqcT[:], WQCT_d[:])
    nc.sync.dma_start(wc[:], WC_d[:])
    nc.sync.dma_start(cto[:], CTO_d[:])

    # ---- UT = wq^T + wqc^T * C  (GPSIMD, bf16 out) ----
    UTt = pool.tile([D, CL], BF16, tag="UTt")
    UT = pool.tile([D, CL], BF16, tag="UT")
    nc.gpsimd.tensor_mul(UTt[:], wqcT[:], Ctile[:])
    nc.gpsimd.tensor_add(UT[:], UTt[:], wqT[:])

    # ---- rbias[c] = sum_d wc * C^T  (DVE, c-part chunks) ----
    rmul = pool.tile([D, NK * D], BF16, tag="rmul")
    rbias = pool.tile([D, NK], F32, tag="rbias")
    cto_v = cto.rearrange("p (k e) -> p k e", k=NK)
    nc.vector.tensor_mul(rmul[:], wc[:], cto_v[:, :, 0:D])
    nc.vector.tensor_reduce(rbias[:], rmul.rearrange("p (k e) -> p k e", k=NK),
                            axis=mybir.AxisListType.X, op=mybir.AluOpType.add)

    # ---- S chunks -> E = exp(S + rbias)  (bf16) ----
    E = pool.tile([D, NK * QL], BF16, tag="E")
    for k in range(NK):
        ps = psS.tile([D, QL], F32, tag="ps")
        nc.tensor.matmul(ps[:], UT[:, k * D:(k + 1) * D], Qb[:],
                         start=True, stop=True)
        nc.scalar.activation(E[:, k * QL:(k + 1) * QL], ps[:], EXP,
                             bias=rbias[:, k:k + 1])

    # ---- F = E^T via DMA xbar transpose (SBUF->SBUF, bf16) ----
    F = pool.tile([D, NJ * CL], BF16, tag="F")
    Fv = F.rearrange("p (j c) -> p j c", j=NJ)
    for k in range(NK):
        eng = nc.scalar if k % 2 == 0 else nc.sync
        eng.dma_start_transpose(Fv[:, :, k * D:(k + 1) * D],
                                E[:, k * QL:(k + 1) * QL])

    # ---- G = E^T @ [C^T|1] per q-chunk; Gn = G/colsum  (bf16) ----
    Gn = pool.tile([D, NJ * D], BF16, tag="Gn")
    crec = pool.tile([D, NJ], F32, tag="crec")
    for j in range(NJ):
        psg = psG.tile([D, D + 1], F32, tag="psg")
        for k in range(NK):
            nc.tensor.matmul(psg[:], E[:, k * QL + j * D: k * QL + (j + 1) * D],
                             cto_v[:, k, :], start=(k == 0), stop=(k == NK - 1))
        nc.vector.reciprocal(crec[:, j:j + 1], psg[:, D:D + 1])
        nc.vector.tensor_scalar_mul(Gn[:, j * D:(j + 1) * D], psg[:, 0:D],
                                    crec[:, j:j + 1])

    # ---- totF row -> rr = 1/totF -> rrB broadcast ----
    rrow = pool.tile([1, CL], F32, tag="rrow")
    rrB = pool.tile([D, CL], F32, tag="rrB")
    for h in range(2):
        psr = psRow.tile([1, QL], F32, tag="psr")
        for j in range(NJ):
            nc.tensor.matmul(psr[:], onesq[:], Fv[:, j, h * QL:(h + 1) * QL],
                             start=(j == 0), stop=(j == NJ - 1))
        nc.vector.reciprocal(rrow[:, h * QL:(h + 1) * QL], psr[:])
    nc.gpsimd.partition_broadcast(rrB[:], rrow[:])

    # ---- A^T, CA, CB + outputs ----
    Asb = pool.tile([D, CL], F32, tag="Asb")
    Bsb = pool.tile([D, CL], F32, tag="Bsb")
    CA = pool.tile([D, CL], F32, tag="CA")
    CB = pool.tile([D, CL], F32, tag="CB")
    for h in range(2):
        hs = slice(h * QL, (h + 1) * QL)
        psa = psAB.tile([D, QL], F32, tag="ps")
        for j in range(NJ):
            nc.tensor.matmul(psa[:], Qt[:, j * D:(j + 1) * D],
                             Fv[:, j, h * QL:(h + 1) * QL],
                             start=(j == 0), stop=(j == NJ - 1))
        nc.vector.tensor_mul(Asb[:, hs], psa[:], rrB[:, hs])
        nc.gpsimd.tensor_mul(CA[:, hs], Asb[:, hs], Ctile[:, hs])
        psb = psAB.tile([D, QL], F32, tag="ps")
        for j in range(NJ):
            nc.tensor.matmul(psb[:], Gn[:, j * D:(j + 1) * D],
                             Fv[:, j, h * QL:(h + 1) * QL],
                             start=(j == 0), stop=(j == NJ - 1))
        nc.vector.tensor_mul(Bsb[:, hs], psb[:], rrB[:, hs])
        nc.gpsimd.tensor_mul(CB[:, hs], Bsb[:, hs], Ctile[:, hs])

    nc.sync.dma_start(OUT_d[0 * D:1 * D, :], Ctile[:])
    nc.sync.dma_start(OUT_d[1 * D:2 * D, :], Asb[:])
    nc.sync.dma_start(OUT_d[2 * D:3 * D, :], CA[:])
    nc.sync.dma_start(OUT_d[3 * D:4 * D, :], CB[:])


def _get_nc():
    global _NC
    if _NC is None:
        _NC = _build()
    return _NC


def _prep_core(Ci, Qi, Wi):
    """Host-side layout prep for one core's shard (pure transposes/casts)."""
    bpc = Ci.shape[0]
    # C^T chunks with ones column: (b, p, k, 129)
    ct = Ci.transpose(0, 2, 1).reshape(bpc, NK, D, D).transpose(0, 2, 1, 3)
    cto = np.concatenate(
        [ct, np.ones((bpc, D, NK, 1), dtype=np.float32)], axis=3)
    # Q^T chunks: (b, qp, j, d)
    qt = Qi.transpose(0, 2, 1).reshape(bpc, NJ, D, D).transpose(0, 2, 1, 3)
    return {
        "C": np.ascontiguousarray(Ci),
        "QB": np.ascontiguousarray(Qi).astype(BF),
        "QT": np.ascontiguousarray(qt.reshape(bpc, D, NJ * D)).astype(BF),
        "WQT": np.ascontiguousarray(
            Wi[:, :, 0:D].transpose(0, 2, 1)).astype(BF),
        "WQCT": np.ascontiguousarray(
            Wi[:, :, 2 * D:3 * D].transpose(0, 2, 1)).astype(BF),
        "WC": np.ascontiguousarray(
            Wi[:, :, D:2 * D].reshape(bpc, NK, D, D).transpose(0, 2, 1, 3)
            .reshape(bpc, D, NK * D)).astype(BF),
        "CTO": np.ascontiguousarray(
            cto.reshape(bpc, D, NK * (D + 1))).astype(BF),
    }


def kernel(C, Q, W):
    C = np.ascontiguousarray(np.asarray(C, dtype=np.float32))
    Q = np.ascontiguousarray(np.asarray(Q, dtype=np.float32))
    W = np.ascontiguousarray(np.asarray(W, dtype=np.float32)).reshape(B, CL, 3 * D)
    in_maps = [
        _prep_core(C[i * BPC:(i + 1) * BPC],
                   Q[i * BPC:(i + 1) * BPC],
                   W[i * BPC:(i + 1) * BPC])
        for i in range(NCORES)
    ]
    nc = _get_nc()
    res = run_bass_kernel_spmd(nc, in_maps, core_ids=list(range(NCORES)), **RUN_KWARGS)
    global LAST_RESULT
    LAST_RESULT = res
    out = np.concatenate([res.results[i]["OUT"] for i in range(NCORES)], axis=0)
    return out


# revision 13
# speedup vs baseline: 1.1266x; 1.1266x over previous
"""CQAttention (BiDAF context-query attention) Trainium2 kernel, v3.

Shapes: C (32,128,1024), Q (32,128,512), W (32768,1,384) -> out (32,512,1024).
Data-parallel across 8 NeuronCores: 4 batches per core, no collectives.

Strategy:
  - All PE matmuls bf16 (2x column rate); zero PE transposes (host supplies
    W^T/Q^T/C^T layouts — pure layout prep).
  - F = exp(S^T) == E^T exactly (the per-context bias r cancels in the row
    normalization), so S^T matmuls + second exp collapse into one DMA xbar
    transpose of E per batch.
  - A^T/B^T scaling row 1/totF via ones-matmul over F; broadcast via
    gpsimd.partition_broadcast; reciprocal taken after broadcast (128 lanes).
  - One packed bf16 input DMA + one C DMA in, one packed f32 DMA out per
    batch (output sections [C | A^T | C*A^T | C*B^T] written in place).
"""

import ml_dtypes
import numpy as np

import concourse.bass as bass
import concourse.bacc as bacc
import concourse.mybir as mybir
from concourse import tile
from concourse.bass_utils import run_bass_kernel_spmd

B, D, CL, QL = 32, 128, 1024, 512
NCORES = 8
BPC = B // NCORES          # batches per core
NK = CL // D               # 8 c-chunks of 128
NJ = QL // D               # 4 q-chunks of 128

# packed bf16 input column offsets
O_QB = 0
O_QT = O_QB + QL
O_WQT = O_QT + NJ * D
O_WQCT = O_WQT + CL
O_WC = O_WQCT + CL
O_CTO = O_WC + NK * D
PKW = O_CTO + NK * (D + 1)   # 5128

F32 = mybir.dt.float32
BF16 = mybir.dt.bfloat16
EXP = mybir.ActivationFunctionType.Exp
BF = ml_dtypes.bfloat16

_NC = None
RUN_KWARGS = {}        # test harness can set e.g. {"trace": True}
LAST_RESULT = None     # last BassKernelResults (for exec_time_ns / trace)


def _build():
    nc = bacc.Bacc("TRN2", debug=False, num_devices=NCORES)

    C_d = nc.dram_tensor("C", [BPC, D, CL], F32, kind="ExternalInput").ap()
    PK_d = nc.dram_tensor("PK", [BPC, D, PKW], BF16, kind="ExternalInput").ap()
    OUT_d = nc.dram_tensor("OUT", [BPC, 4 * D, CL], F32, kind="ExternalOutput").ap()

    with tile.TileContext(nc) as tc:
        with (
            tc.tile_pool(name="const", bufs=1) as cpool,
            tc.tile_pool(name="work", bufs=2) as pool,
            tc.tile_pool(name="psS", bufs=3, space="PSUM") as psS,
            tc.tile_pool(name="psG", bufs=2, space="PSUM") as psG,
            tc.tile_pool(name="psAB", bufs=2, space="PSUM") as psAB,
            tc.tile_pool(name="psRow", bufs=1, space="PSUM") as psRow,
        ):
            onesq = cpool.tile([D, 1], BF16)
            nc.vector.memset(onesq[:], 1.0)
            pools = (pool, psS, psG, psAB, psRow)
            for b in range(BPC):
                _batch(nc, tc, pools, onesq, C_d[b], PK_d[b], OUT_d[b])
    nc.compile()
    return nc


def _batch(nc, tc, pools, onesq, C_d, PK_d, OUT_d):
    pool, psS, psG, psAB, psRow = pools

    # ---- input loads ----
    OB = pool.tile([D, 4 * CL], F32, tag="OB")      # [C | Asb | CA | CB]
    pk = pool.tile([D, PKW], BF16, tag="pk")
    Ctile = OB[:, 0:CL]
    nc.sync.dma_start(Ctile, C_d[:])
    nc.sync.dma_start(pk[:], PK_d[:])
    Qb = pk[:, O_QB:O_QB + QL]
    Qt = pk[:, O_QT:O_QT + NJ * D]
    wqT = pk[:, O_WQT:O_WQT + CL]
    wqcT = pk[:, O_WQCT:O_WQCT + CL]
    wc = pk[:, O_WC:O_WC + NK * D]
    cto = pk[:, O_CTO:O_CTO + NK * (D + 1)]
    cto_v = cto.rearrange("p (k e) -> p k e", k=NK)

    # ---- UT = wq^T + wqc^T * C  (GPSIMD, bf16 out) ----
    UTt = pool.tile([D, CL], BF16, tag="UTt")
    UT = pool.tile([D, CL], BF16, tag="UT")
    nc.gpsimd.tensor_mul(UTt[:], wqcT[:], Ctile)
    nc.gpsimd.tensor_add(UT[:], UTt[:], wqT[:])

    # ---- rbias[c] = sum_d wc * C^T  (DVE, c-part chunks) ----
    rmul = pool.tile([D, NK * D], BF16, tag="rmul")
    rbias = pool.tile([D, NK], F32, tag="rbias")
    nc.vector.tensor_mul(rmul[:], wc[:], cto_v[:, :, 0:D])
    nc.vector.tensor_reduce(rbias[:], rmul.rearrange("p (k e) -> p k e", k=NK),
                            axis=mybir.AxisListType.X, op=mybir.AluOpType.add)

    # ---- S chunks -> E = exp(S + rbias)  (bf16) ----
    E = pool.tile([D, NK * QL], BF16, tag="E")
    for k in range(NK):
        ps = psS.tile([D, QL], F32, tag="ps")
        nc.tensor.matmul(ps[:], UT[:, k * D:(k + 1) * D], Qb[:],
                         start=True, stop=True)
        nc.scalar.activation(E[:, k * QL:(k + 1) * QL], ps[:], EXP,
                             bias=rbias[:, k:k + 1])

    # ---- F = E^T in one DMA xbar transpose (SBUF->SBUF, bf16) ----
    # E free index r = k*512 + j*128 + qp ; F stored k-major:
    # F col = k*512 + j*128 + cp, so transposed block di = r//128 = k*4+j
    # lands at F cols di*128..di*128+128 — a plain 3D out AP.
    F = pool.tile([D, NJ * CL], BF16, tag="F")
    Fr = F.rearrange("p (k j c) -> p k j c", k=NK, j=NJ)
    nc.scalar.dma_start_transpose(
        F.rearrange("p (m c) -> p m c", m=NK * NJ), E[:])

    # ---- G = E^T @ [C^T|1] per q-chunk; Gn = G/colsum  (bf16) ----
    Gn = pool.tile([D, NJ * D], BF16, tag="Gn")
    crec = pool.tile([D, NJ], F32, tag="crec")
    for j in range(NJ):
        psg = psG.tile([D, D + 1], F32, tag="psg")
        for k in range(NK):
            nc.tensor.matmul(psg[:], E[:, k * QL + j * D: k * QL + (j + 1) * D],
                             cto_v[:, k, :], start=(k == 0), stop=(k == NK - 1))
        nc.vector.reciprocal(crec[:, j:j + 1], psg[:, D:D + 1])
        nc.vector.tensor_scalar_mul(Gn[:, j * D:(j + 1) * D], psg[:, 0:D],
                                    crec[:, j:j + 1])

    # ---- totF row -> broadcast -> rrB = 1/totF_bcast ----
    trow = pool.tile([1, CL], F32, tag="trow")
    rrB0 = pool.tile([D, CL], F32, tag="rrB0")
    rrB = pool.tile([D, CL], F32, tag="rrB")
    CrrB = pool.tile([D, CL], F32, tag="CrrB")
    for h in range(2):
        psr = psRow.tile([1, QL], F32, tag="psr")
        for j in range(NJ):
            nc.tensor.matmul(psr[:], onesq[:], Fr[:, 4 * h:4 * h + 4, j, :],
                             start=(j == 0), stop=(j == NJ - 1))
        nc.scalar.copy(trow[:, h * QL:(h + 1) * QL], psr[:])
    nc.gpsimd.partition_broadcast(rrB0[:], trow[:])
    nc.vector.reciprocal(rrB[:], rrB0[:])
    nc.gpsimd.tensor_mul(CrrB[:], Ctile, rrB[:])

    # ---- A^T, CA, CB into output sections ----
    Asb = OB[:, CL:2 * CL]
    CA = OB[:, 2 * CL:3 * CL]
    CB = OB[:, 3 * CL:4 * CL]
    for h in range(2):
        hs = slice(h * QL, (h + 1) * QL)
        psa = psAB.tile([D, QL], F32, tag="ps")
        for j in range(NJ):
            nc.tensor.matmul(psa[:], Qt[:, j * D:(j + 1) * D],
                             Fr[:, 4 * h:4 * h + 4, j, :],
                             start=(j == 0), stop=(j == NJ - 1))
        nc.vector.tensor_mul(Asb[:, hs], psa[:], rrB[:, hs])
        nc.vector.tensor_mul(CA[:, hs], psa[:], CrrB[:, hs])
        psb = psAB.tile([D, QL], F32, tag="ps")
        for j in range(NJ):
            nc.tensor.matmul(psb[:], Gn[:, j * D:(j + 1) * D],
                             Fr[:, 4 * h:4 * h + 4, j, :],
                             start=(j == 0), stop=(j == NJ - 1))
        nc.vector.tensor_mul(CB[:, hs], psb[:], CrrB[:, hs])

    nc.sync.dma_start(OUT_d.rearrange("(r p) c -> p r c", p=D),
                      OB.rearrange("p (r c) -> p r c", r=4))


def _get_nc():
    global _NC
    if _NC is None:
        _NC = _build()
    return _NC


def _prep_core(Ci, Qi, Wi):
    """Host-side layout prep for one core's shard (pure transposes/casts)."""
    bpc = Ci.shape[0]
    pk = np.empty((bpc, D, PKW), dtype=BF)
    pk[:, :, O_QB:O_QB + QL] = Qi.astype(BF)
    qt = Qi.transpose(0, 2, 1).reshape(bpc, NJ, D, D).transpose(0, 2, 1, 3)
    pk[:, :, O_QT:O_QT + NJ * D] = qt.reshape(bpc, D, NJ * D).astype(BF)
    pk[:, :, O_WQT:O_WQT + CL] = Wi[:, :, 0:D].transpose(0, 2, 1).astype(BF)
    pk[:, :, O_WQCT:O_WQCT + CL] = (
        Wi[:, :, 2 * D:3 * D].transpose(0, 2, 1).astype(BF))
    pk[:, :, O_WC:O_WC + NK * D] = (
        Wi[:, :, D:2 * D].reshape(bpc, NK, D, D).transpose(0, 2, 1, 3)
        .reshape(bpc, D, NK * D).astype(BF))
    ct = Ci.transpose(0, 2, 1).reshape(bpc, NK, D, D).transpose(0, 2, 1, 3)
    cto = np.concatenate(
        [ct, np.ones((bpc, D, NK, 1), dtype=np.float32)], axis=3)
    pk[:, :, O_CTO:O_CTO + NK * (D + 1)] = (
        cto.reshape(bpc, D, NK * (D + 1)).astype(BF))
    return {"C": np.ascontiguousarray(Ci), "PK": pk}


def kernel(C, Q, W):
    C = np.ascontiguousarray(np.asarray(C, dtype=np.float32))
    Q = np.ascontiguousarray(np.asarray(Q, dtype=np.float32))
    W = np.ascontiguousarray(np.asarray(W, dtype=np.float32)).reshape(B, CL, 3 * D)
    in_maps = [
        _prep_core(C[i * BPC:(i + 1) * BPC],
                   Q[i * BPC:(i + 1) * BPC],
                   W[i * BPC:(i + 1) * BPC])
        for i in range(NCORES)
    ]
    nc = _get_nc()
    res = run_bass_kernel_spmd(nc, in_maps, core_ids=list(range(NCORES)), **RUN_KWARGS)
    global LAST_RESULT
    LAST_RESULT = res
    out = np.concatenate([res.results[i]["OUT"] for i in range(NCORES)], axis=0)
    return out


# revision 15
# speedup vs baseline: 1.1919x; 1.0579x over previous
"""CQAttention (BiDAF context-query attention) Trainium2 kernel, v4.

Shapes: C (32,128,1024), Q (32,128,512), W (32768,1,384) -> out (32,512,1024).
Data-parallel across 8 NeuronCores: 4 batches per core, no collectives.

Strategy:
  - All PE matmuls bf16 (2x column rate); zero PE transposes (host supplies
    W^T/Q^T/C^T layouts — pure layout prep).
  - F = exp(S^T) == E^T exactly (the per-context bias r cancels in the row
    normalization), so S^T matmuls + second exp collapse into two DMA xbar
    transposes of E halves per batch.
  - Row normalizer 1/sum_q exp(S) computed c-partition-wise from the ACT
    accumulator (tiny (128,8) ops), broadcast via DRAM bounce.
  - All input DMAs for all batches issued up front into dedicated tiles so
    the FIFO HWDGE ring never stalls input prefetch behind output waits.
  - sync ring: bulk in / xbar / out.  scalar ring: ACT + tiny DMAs.
"""

import ml_dtypes
import numpy as np

import concourse.bass as bass
import concourse.bacc as bacc
import concourse.mybir as mybir
from concourse import tile
from concourse.bass_utils import run_bass_kernel_spmd

B, D, CL, QL = 32, 128, 1024, 512
NCORES = 8
BPC = B // NCORES          # batches per core
NK = CL // D               # 8 c-chunks of 128
NJ = QL // D               # 4 q-chunks of 128

# packed bf16 input column offsets
O_QB = 0
O_QT = O_QB + QL
O_WQT = O_QT + NJ * D
O_WQCT = O_WQT + CL
O_WC = O_WQCT + CL
O_CTO = O_WC + NK * D
PKW = O_CTO + NK * (D + 1)   # 5128

F32 = mybir.dt.float32
BF16 = mybir.dt.bfloat16
EXP = mybir.ActivationFunctionType.Exp
BF = ml_dtypes.bfloat16

_NC = None
RUN_KWARGS = {}        # test harness can set e.g. {"trace": True}
LAST_RESULT = None     # last BassKernelResults (for exec_time_ns / trace)


def _build():
    nc = bacc.Bacc("TRN2", debug=False, num_devices=NCORES)

    C_d = nc.dram_tensor("C", [BPC, D, CL], F32, kind="ExternalInput").ap()
    PK_d = nc.dram_tensor("PK", [BPC, D, PKW], BF16, kind="ExternalInput").ap()
    OUT_d = nc.dram_tensor("OUT", [BPC, 4 * D, CL], F32, kind="ExternalOutput").ap()

    with tile.TileContext(nc) as tc:
        with (
            tc.tile_pool(name="ins", bufs=1) as ipool,
            tc.tile_pool(name="work", bufs=2) as pool,
            tc.tile_pool(name="psS", bufs=3, space="PSUM") as psS,
            tc.tile_pool(name="psG", bufs=2, space="PSUM") as psG,
            tc.tile_pool(name="psAB", bufs=3, space="PSUM") as psAB,
            tc.tile_pool(name="dram", bufs=2, space="DRAM") as dram,
        ):
            # all input loads up front (dedicated tiles, no FIFO stalls)
            OBs, pks = [], []
            for b in range(BPC):
                OB = ipool.tile([D, 4 * CL], F32, tag=f"OB{b}")
                pk = ipool.tile([D, PKW], BF16, tag=f"pk{b}")
                nc.sync.dma_start(OB[:, 0:CL], C_d[b])
                nc.sync.dma_start(pk[:], PK_d[b])
                OBs.append(OB)
                pks.append(pk)
            pools = (pool, psS, psG, psAB, dram)
            for b in range(BPC):
                _batch(nc, tc, pools, OBs[b], pks[b], OUT_d[b])
    nc.compile()
    return nc


def _batch(nc, tc, pools, OB, pk, OUT_d):
    pool, psS, psG, psAB, dram = pools

    Ctile = OB[:, 0:CL]
    Qb = pk[:, O_QB:O_QB + QL]
    Qt = pk[:, O_QT:O_QT + NJ * D]
    wqT = pk[:, O_WQT:O_WQT + CL]
    wqcT = pk[:, O_WQCT:O_WQCT + CL]
    wc = pk[:, O_WC:O_WC + NK * D]
    cto = pk[:, O_CTO:O_CTO + NK * (D + 1)]
    cto_v = cto.rearrange("p (k e) -> p k e", k=NK)

    # ---- UT = wq^T + wqc^T * C  (GPSIMD, bf16 out) ----
    UTt = pool.tile([D, CL], BF16, tag="UTt")
    UT = pool.tile([D, CL], BF16, tag="UT")
    nc.gpsimd.tensor_mul(UTt[:], wqcT[:], Ctile)
    nc.gpsimd.tensor_add(UT[:], UTt[:], wqT[:])

    # ---- rbias[c] = sum_d wc * C^T  (DVE, c-part chunks) ----
    rmul = pool.tile([D, NK * D], BF16, tag="rmul")
    rbias = pool.tile([D, NK], F32, tag="rbias")
    nc.vector.tensor_mul(rmul[:], wc[:], cto_v[:, :, 0:D])
    nc.vector.tensor_reduce(rbias[:], rmul.rearrange("p (k e) -> p k e", k=NK),
                            axis=mybir.AxisListType.X, op=mybir.AluOpType.add)

    # ---- S chunks -> E = exp(S + rbias), rowsum accum  (bf16) ----
    E = pool.tile([D, NK * QL], BF16, tag="E")
    rowsum = pool.tile([D, NK], F32, tag="rowsum")
    for k in range(NK):
        ps = psS.tile([D, QL], F32, tag="ps")
        nc.tensor.matmul(ps[:], UT[:, k * D:(k + 1) * D], Qb[:],
                         start=True, stop=True)
        nc.scalar.activation(E[:, k * QL:(k + 1) * QL], ps[:], EXP,
                             bias=rbias[:, k:k + 1],
                             accum_out=rowsum[:, k:k + 1])

    # ---- F = E^T via DMA xbar transpose, per half (SBUF->SBUF, bf16) ----
    # E free index r = k*512 + j*128 + qp ; F stored k-major:
    # F col = k*512 + j*128 + cp -> transposed block di = r//128 lands at
    # F cols di*128.., a plain 3D out AP per half.
    F = pool.tile([D, NJ * CL], BF16, tag="F")
    Fr = F.rearrange("p (k j c) -> p k j c", k=NK, j=NJ)
    Fh = F.rearrange("p (h m c) -> p h m c", h=2, m=NK * NJ // 2)
    for h in range(2):
        nc.sync.dma_start_transpose(
            Fh[:, h, :, :], E[:, h * 4 * QL:(h + 1) * 4 * QL])

    # ---- scale = 1/rowsum (F = E^T carries exp(r), so no er factor)
    #      -> rrB broadcast via DRAM bounce ----
    rsi = pool.tile([D, NK], F32, tag="rsi")
    rrB = pool.tile([D, CL], F32, tag="rrB")
    CrrB = pool.tile([D, CL], F32, tag="CrrB")
    nc.vector.reciprocal(rsi[:], rowsum[:])
    scratch = dram.tile([NK, D], F32, tag="scratch")
    nc.scalar.dma_start(scratch.rearrange("k p -> p k"), rsi[:])
    nc.scalar.dma_start(
        rrB[:], scratch.rearrange("k p -> (k p)")[None, :].partition_broadcast(D))
    nc.gpsimd.tensor_mul(CrrB[:], Ctile, rrB[:])

    # ---- G = E^T @ [C^T|1] per q-chunk; Gn = G/colsum  (bf16) ----
    Gn = pool.tile([D, NJ * D], BF16, tag="Gn")
    crec = pool.tile([D, NJ], F32, tag="crec")
    for j in range(NJ):
        psg = psG.tile([D, D + 1], F32, tag="psg")
        for k in range(NK):
            nc.tensor.matmul(psg[:], E[:, k * QL + j * D: k * QL + (j + 1) * D],
                             cto_v[:, k, :], start=(k == 0), stop=(k == NK - 1))
        nc.vector.reciprocal(crec[:, j:j + 1], psg[:, D:D + 1])
        nc.vector.tensor_scalar_mul(Gn[:, j * D:(j + 1) * D], psg[:, 0:D],
                                    crec[:, j:j + 1])

    # ---- A^T, CA, CB into output sections ----
    Asb = OB[:, CL:2 * CL]
    CA = OB[:, 2 * CL:3 * CL]
    CB = OB[:, 3 * CL:4 * CL]
    for h in range(2):
        hs = slice(h * QL, (h + 1) * QL)
        psa = psAB.tile([D, QL], F32, tag="ps")
        for j in range(NJ):
            nc.tensor.matmul(psa[:], Qt[:, j * D:(j + 1) * D],
                             Fr[:, 4 * h:4 * h + 4, j, :],
                             start=(j == 0), stop=(j == NJ - 1))
        nc.vector.tensor_mul(Asb[:, hs], psa[:], rrB[:, hs])
        nc.vector.tensor_mul(CA[:, hs], psa[:], CrrB[:, hs])
        psb = psAB.tile([D, QL], F32, tag="ps")
        for j in range(NJ):
            nc.tensor.matmul(psb[:], Gn[:, j * D:(j + 1) * D],
                             Fr[:, 4 * h:4 * h + 4, j, :],
                             start=(j == 0), stop=(j == NJ - 1))
        nc.vector.tensor_mul(CB[:, hs], psb[:], CrrB[:, hs])

    nc.sync.dma_start(OUT_d.rearrange("(r p) c -> p r c", p=D),
                      OB.rearrange("p (r c) -> p r c", r=4))


def _get_nc():
    global _NC
    if _NC is None:
        _NC = _build()
    return _NC


def _prep_core(Ci, Qi, Wi):
    """Host-side layout prep for one core's shard (pure transposes/casts)."""
    bpc = Ci.shape[0]
    pk = np.empty((bpc, D, PKW), dtype=BF)
    pk[:, :, O_QB:O_QB + QL] = Qi.astype(BF)
    qt = Qi.transpose(0, 2, 1).reshape(bpc, NJ, D, D).transpose(0, 2, 1, 3)
    pk[:, :, O_QT:O_QT + NJ * D] = qt.reshape(bpc, D, NJ * D).astype(BF)
    pk[:, :, O_WQT:O_WQT + CL] = Wi[:, :, 0:D].transpose(0, 2, 1).astype(BF)
    pk[:, :, O_WQCT:O_WQCT + CL] = (
        Wi[:, :, 2 * D:3 * D].transpose(0, 2, 1).astype(BF))
    pk[:, :, O_WC:O_WC + NK * D] = (
        Wi[:, :, D:2 * D].reshape(bpc, NK, D, D).transpose(0, 2, 1, 3)
        .reshape(bpc, D, NK * D).astype(BF))
    ct = Ci.transpose(0, 2, 1).reshape(bpc, NK, D, D).transpose(0, 2, 1, 3)
    cto = np.concatenate(
        [ct, np.ones((bpc, D, NK, 1), dtype=np.float32)], axis=3)
    pk[:, :, O_CTO:O_CTO + NK * (D + 1)] = (
        cto.reshape(bpc, D, NK * (D + 1)).astype(BF))
    return {"C": np.ascontiguousarray(Ci), "PK": pk}


def kernel(C, Q, W):
    C = np.ascontiguousarray(np.asarray(C, dtype=np.float32))
    Q = np.ascontiguousarray(np.asarray(Q, dtype=np.float32))
    W = np.ascontiguousarray(np.asarray(W, dtype=np.float32)).reshape(B, CL, 3 * D)
    in_maps = [
        _prep_core(C[i * BPC:(i + 1) * BPC],
                   Q[i * BPC:(i + 1) * BPC],
                   W[i * BPC:(i + 1) * BPC])
        for i in range(NCORES)
    ]
    nc = _get_nc()
    res = run_bass_kernel_spmd(nc, in_maps, core_ids=list(range(NCORES)), **RUN_KWARGS)
    global LAST_RESULT
    LAST_RESULT = res
    out = np.concatenate([res.results[i]["OUT"] for i in range(NCORES)], axis=0)
    return out


# revision 16
# speedup vs baseline: 1.3074x; 1.0969x over previous
"""CQAttention (BiDAF context-query attention) Trainium2 kernel, v5.

Shapes: C (32,128,1024), Q (32,128,512), W (32768,1,384) -> out (32,512,1024).
Data-parallel across 8 NeuronCores: 4 batches per core, no collectives.

Strategy:
  - All PE matmuls bf16; zero PE transposes (host supplies W^T/Q^T/C^T
    layouts — pure layout prep).
  - F = exp(S^T) == E^T exactly (the per-context bias r cancels in the row
    normalization): S^T matmuls + second exp collapse into two DMA xbar
    transposes of E halves; A/B scale is then just 1/rowsum.
  - Software-pipelined emission so strict-FIFO engine queues never hold
    next-batch early work behind this-batch late work:
      upfront:  all input DMAs; UT (gpsimd); rbias (DVE) for all batches
      stage1(b): S matmuls + exp->E (+rowsum accum) + 1/rowsum bounce
      stage2(b): xbar E->F, G matmuls + Gn
      stage3(b): A/B matmuls, scaling (DVE), C*A/C*B (gpsimd), output DMA
    emitted as s1(0) s2(0) s1(1) s2(1) s3(0) s1(2) s2(2) s3(1) ...
"""

import ml_dtypes
import numpy as np

import concourse.bass as bass
import concourse.bacc as bacc
import concourse.mybir as mybir
from concourse import tile
from concourse.bass_utils import run_bass_kernel_spmd

B, D, CL, QL = 32, 128, 1024, 512
NCORES = 8
BPC = B // NCORES          # batches per core
NK = CL // D               # 8 c-chunks of 128
NJ = QL // D               # 4 q-chunks of 128

# packed bf16 input column offsets
O_QB = 0
O_QT = O_QB + QL
O_WQT = O_QT + NJ * D
O_WQCT = O_WQT + CL
O_WC = O_WQCT + CL
O_CTO = O_WC + NK * D
PKW = O_CTO + NK * (D + 1)   # 5128

F32 = mybir.dt.float32
BF16 = mybir.dt.bfloat16
EXP = mybir.ActivationFunctionType.Exp
BF = ml_dtypes.bfloat16

_NC = None
RUN_KWARGS = {}        # test harness can set e.g. {"trace": True}
LAST_RESULT = None     # last BassKernelResults (for exec_time_ns / trace)


class _Batch:
    """Per-batch tiles and views."""

    def __init__(self, nc, ipool, pool, dram, b, C_d, PK_d, OUT_d):
        self.OUT_d = OUT_d[b]
        self.OB = ipool.tile([D, 4 * CL], F32, tag=f"OB{b}")
        self.pk = ipool.tile([D, PKW], BF16, tag=f"pk{b}")
        nc.sync.dma_start(self.OB[:, 0:CL], C_d[b])
        nc.sync.dma_start(self.pk[:], PK_d[b])
        self.C = self.OB[:, 0:CL]
        self.Qb = self.pk[:, O_QB:O_QB + QL]
        self.Qt = self.pk[:, O_QT:O_QT + NJ * D]
        self.wqT = self.pk[:, O_WQT:O_WQT + CL]
        self.wqcT = self.pk[:, O_WQCT:O_WQCT + CL]
        self.wc = self.pk[:, O_WC:O_WC + NK * D]
        cto = self.pk[:, O_CTO:O_CTO + NK * (D + 1)]
        self.cto_v = cto.rearrange("p (k e) -> p k e", k=NK)
        self.UTt = ipool.tile([D, CL], BF16, tag=f"UTt{b}")
        self.UT = ipool.tile([D, CL], BF16, tag=f"UT{b}")
        self.rmul = ipool.tile([D, NK * D], BF16, tag=f"rmul{b}")
        self.rbias = ipool.tile([D, NK], F32, tag=f"rbias{b}")
        # pooled (bufs=2) per-batch working tiles
        self.E = pool.tile([D, NK * QL], BF16, tag="E")
        self.F = pool.tile([D, NJ * CL], BF16, tag="F")
        self.Fr = self.F.rearrange("p (k j c) -> p k j c", k=NK, j=NJ)
        self.Fh = self.F.rearrange("p (h m c) -> p h m c", h=2, m=NK * NJ // 2)
        self.rowsum = pool.tile([D, NK], F32, tag="rowsum")
        self.rsi = pool.tile([D, NK], F32, tag="rsi")
        self.rrB = pool.tile([D, CL], F32, tag="rrB")
        self.Gn = pool.tile([D, NJ * D], BF16, tag="Gn")
        self.crec = pool.tile([D, NJ], F32, tag="crec")
        self.Bsb = pool.tile([D, CL], F32, tag="Bsb")
        self.scratch = dram.tile([NK, D], F32, tag="scratch")


def _upfront(nc, t):
    # UT = wq^T + wqc^T * C  (GPSIMD, bf16 out) — batch-independent
    nc.gpsimd.tensor_mul(t.UTt[:], t.wqcT[:], t.C)
    nc.gpsimd.tensor_add(t.UT[:], t.UTt[:], t.wqT[:])
    # rbias[c] = sum_d wc * C^T  (DVE, c-part chunks)
    nc.vector.tensor_mul(t.rmul[:], t.wc[:], t.cto_v[:, :, 0:D])
    nc.vector.tensor_reduce(t.rbias[:],
                            t.rmul.rearrange("p (k e) -> p k e", k=NK),
                            axis=mybir.AxisListType.X, op=mybir.AluOpType.add)


def _stage1(nc, t, psS):
    # S chunks -> E = exp(S + rbias), rowsum accum (bf16)
    for k in range(NK):
        ps = psS.tile([D, QL], F32, tag="ps")
        nc.tensor.matmul(ps[:], t.UT[:, k * D:(k + 1) * D], t.Qb[:],
                         start=True, stop=True)
        nc.scalar.activation(t.E[:, k * QL:(k + 1) * QL], ps[:], EXP,
                             bias=t.rbias[:, k:k + 1],
                             accum_out=t.rowsum[:, k:k + 1])
    # scale = 1/rowsum -> broadcast via DRAM bounce (scalar ring)
    nc.vector.reciprocal(t.rsi[:], t.rowsum[:])
    nc.scalar.dma_start(t.scratch.rearrange("k p -> p k"), t.rsi[:])
    nc.scalar.dma_start(
        t.rrB[:],
        t.scratch.rearrange("k p -> (k p)")[None, :].partition_broadcast(D))


def _stage2(nc, t, psG):
    # F = E^T via DMA xbar transpose per half (sync ring)
    for h in range(2):
        nc.sync.dma_start_transpose(
            t.Fh[:, h, :, :], t.E[:, h * 4 * QL:(h + 1) * 4 * QL])
    # G = E^T @ [C^T|1] per q-chunk; Gn = G/colsum (bf16)
    for j in range(NJ):
        psg = psG.tile([D, D + 1], F32, tag="psg")
        for k in range(NK):
            nc.tensor.matmul(psg[:],
                             t.E[:, k * QL + j * D: k * QL + (j + 1) * D],
                             t.cto_v[:, k, :],
                             start=(k == 0), stop=(k == NK - 1))
        nc.vector.reciprocal(t.crec[:, j:j + 1], psg[:, D:D + 1])
        nc.vector.tensor_scalar_mul(t.Gn[:, j * D:(j + 1) * D], psg[:, 0:D],
                                    t.crec[:, j:j + 1])


def _stage3(nc, t, psAB):
    Asb = t.OB[:, CL:2 * CL]
    CA = t.OB[:, 2 * CL:3 * CL]
    CB = t.OB[:, 3 * CL:4 * CL]
    for h in range(2):
        hs = slice(h * QL, (h + 1) * QL)
        psa = psAB.tile([D, QL], F32, tag="ps")
        for j in range(NJ):
            nc.tensor.matmul(psa[:], t.Qt[:, j * D:(j + 1) * D],
                             t.Fr[:, 4 * h:4 * h + 4, j, :],
                             start=(j == 0), stop=(j == NJ - 1))
        nc.vector.tensor_mul(Asb[:, hs], psa[:], t.rrB[:, hs])
        nc.gpsimd.tensor_mul(CA[:, hs], Asb[:, hs], t.C[:, hs])
        psb = psAB.tile([D, QL], F32, tag="ps")
        for j in range(NJ):
            nc.tensor.matmul(psb[:], t.Gn[:, j * D:(j + 1) * D],
                             t.Fr[:, 4 * h:4 * h + 4, j, :],
                             start=(j == 0), stop=(j == NJ - 1))
        nc.vector.tensor_mul(t.Bsb[:, hs], psb[:], t.rrB[:, hs])
        nc.gpsimd.tensor_mul(CB[:, hs], t.Bsb[:, hs], t.C[:, hs])
    nc.sync.dma_start(t.OUT_d.rearrange("(r p) c -> p r c", p=D),
                      t.OB.rearrange("p (r c) -> p r c", r=4))


def _build():
    nc = bacc.Bacc("TRN2", debug=False, num_devices=NCORES)

    C_d = nc.dram_tensor("C", [BPC, D, CL], F32, kind="ExternalInput").ap()
    PK_d = nc.dram_tensor("PK", [BPC, D, PKW], BF16, kind="ExternalInput").ap()
    OUT_d = nc.dram_tensor("OUT", [BPC, 4 * D, CL], F32, kind="ExternalOutput").ap()

    with tile.TileContext(nc) as tc:
        with (
            tc.tile_pool(name="ins", bufs=1) as ipool,
            tc.tile_pool(name="work", bufs=2) as pool,
            tc.tile_pool(name="psS", bufs=3, space="PSUM") as psS,
            tc.tile_pool(name="psG", bufs=2, space="PSUM") as psG,
            tc.tile_pool(name="psAB", bufs=3, space="PSUM") as psAB,
            tc.tile_pool(name="dram", bufs=2, space="DRAM") as dram,
        ):
            ts = [_Batch(nc, ipool, pool, dram, b, C_d, PK_d, OUT_d)
                  for b in range(BPC)]
            for t in ts:
                _upfront(nc, t)
            # software-pipelined emission:
            # s1(0) s2(0) s1(1) s2(1) s3(0) s1(2) s2(2) s3(1) s1(3) s2(3)
            # s3(2) s3(3)
            _stage1(nc, ts[0], psS)
            _stage2(nc, ts[0], psG)
            for b in range(1, BPC):
                _stage1(nc, ts[b], psS)
                _stage2(nc, ts[b], psG)
                _stage3(nc, ts[b - 1], psAB)
            _stage3(nc, ts[BPC - 1], psAB)
    nc.compile()
    return nc


def _get_nc():
    global _NC
    if _NC is None:
        _NC = _build()
    return _NC


def _prep_core(Ci, Qi, Wi):
    """Host-side layout prep for one core's shard (pure transposes/casts)."""
    bpc = Ci.shape[0]
    pk = np.empty((bpc, D, PKW), dtype=BF)
    pk[:, :, O_QB:O_QB + QL] = Qi.astype(BF)
    qt = Qi.transpose(0, 2, 1).reshape(bpc, NJ, D, D).transpose(0, 2, 1, 3)
    pk[:, :, O_QT:O_QT + NJ * D] = qt.reshape(bpc, D, NJ * D).astype(BF)
    pk[:, :, O_WQT:O_WQT + CL] = Wi[:, :, 0:D].transpose(0, 2, 1).astype(BF)
    pk[:, :, O_WQCT:O_WQCT + CL] = (
        Wi[:, :, 2 * D:3 * D].transpose(0, 2, 1).astype(BF))
    pk[:, :, O_WC:O_WC + NK * D] = (
        Wi[:, :, D:2 * D].reshape(bpc, NK, D, D).transpose(0, 2, 1, 3)
        .reshape(bpc, D, NK * D).astype(BF))
    ct = Ci.transpose(0, 2, 1).reshape(bpc, NK, D, D).transpose(0, 2, 1, 3)
    cto = np.concatenate(
        [ct, np.ones((bpc, D, NK, 1), dtype=np.float32)], axis=3)
    pk[:, :, O_CTO:O_CTO + NK * (D + 1)] = (
        cto.reshape(bpc, D, NK * (D + 1)).astype(BF))
    return {"C": np.ascontiguousarray(Ci), "PK": pk}


def kernel(C, Q, W):
    C = np.ascontiguousarray(np.asarray(C, dtype=np.float32))
    Q = np.ascontiguousarray(np.asarray(Q, dtype=np.float32))
    W = np.ascontiguousarray(np.asarray(W, dtype=np.float32)).reshape(B, CL, 3 * D)
    in_maps = [
        _prep_core(C[i * BPC:(i + 1) * BPC],
                   Q[i * BPC:(i + 1) * BPC],
                   W[i * BPC:(i + 1) * BPC])
        for i in range(NCORES)
    ]
    nc = _get_nc()
    res = run_bass_kernel_spmd(nc, in_maps, core_ids=list(range(NCORES)), **RUN_KWARGS)
    global LAST_RESULT
    LAST_RESULT = res
    out = np.concatenate([res.results[i]["OUT"] for i in range(NCORES)], axis=0)
    return out
